# revision 21
# baseline (speedup 1.0000x reference)
"""Trainium2 Bass kernel for nn_Encoding3D (vq_codebook encoding layer).

Computes, for X (B,C,D,H,W), codewords (K,C), scale (K,):
    logits[b,n,k] = scale_k * (|x_bn|^2 + |c_k|^2 - 2 x_bn.c_k)
    A = softmax_k(logits);  coefA = A^T reshaped (B,K,D,H,W)
    E[b,k,:] = sum_n A[b,n,k] * (x_bn - c_k)
Returns (E, coefA) like the reference.

Sharding: 8 cores, each handles half of one batch's N = D*H*W positions.
Device pipeline per core (positions in P-tiles of 2048 = 4 chunks x 512):
  - 3 accumulating bf16 matmuls produce packed-KN logits psum
    P[(jc,k), f] = scale_k*(X2 - 2 XC) (codebook/scale folded into weights;
    X2 precomputed on host, split hi/lo bf16 for precision, injected via a
    rank-8 matmul)
  - ACT exp with per-partition bias scale_k*|c_k|^2 -> U (unnormalized
    softmax numerator, bf16, exp<=0 args so no overflow; max-subtraction
    is unnecessary since max logit ~ -few, verified vs reference)
  - PE transposes U into NK layout; DVE computes Z (sum over k), 1/Z
  - X cast to bf16, PE-transposed to X^T tiles; AtX accumulated over the
    whole shard into one persistent PSUM tile via matmuls
    lhsT = A_nk-slab (128f x 128(jc,k)), rhs = X^T-slab (128f x 256(jc,c))
  - outputs: U (packed KN, bf16), Z (f32), AtX partials (f32)
Host: divides U by Z (coefA), sums Asum from coefA, E = AtX - Asum*cw.
"""

import os
import sys

sys.path.insert(0, "/opt/trn_rl_repo")

import numpy as np
import ml_dtypes

import concourse.bass as bass
import concourse.bacc as bacc
import concourse.tile as tile
from concourse import mybir
from concourse.bass_utils import run_bass_kernel_spmd

# Problem dims (hardcoded per contract)
B, C, D, H, W = 4, 64, 32, 64, 64
K = 32
N = D * H * W  # 131072
NCORES = 8
NSH = B * N // NCORES  # 65536 positions per core (half a batch)
FC = 512  # psum free columns
PPT = 4 * FC  # positions per P-tile (4 chunks of FC)
PT = NSH // PPT  # 32 P-tiles

f32 = mybir.dt.float32
f32r = mybir.dt.float32r
bf16 = mybir.dt.bfloat16

_COMPILED = {}

last_exec_time_ns = None


def _build_program(reps=1, variant="full"):
    nc = bacc.Bacc("TRN2", target_bir_lowering=False, debug=False)

    xs = nc.dram_tensor("xs", [C, NSH], f32, kind="ExternalInput").ap()
    x2s = nc.dram_tensor("x2s", [2, NSH], bf16, kind="ExternalInput").ap()
    la_d = nc.dram_tensor("la", [128, 128], bf16, kind="ExternalInput").ap()
    lb_d = nc.dram_tensor("lb", [128, 128], bf16, kind="ExternalInput").ap()
    lx2_d = nc.dram_tensor("lx2", [8, 128], bf16, kind="ExternalInput").ap()
    bias_d = nc.dram_tensor("bias", [128, 1], f32, kind="ExternalInput").ap()
    id_d = nc.dram_tensor("ident", [128, 128], bf16, kind="ExternalInput").ap()

    ub = nc.dram_tensor("ub", [128, PT * FC], bf16, kind="ExternalOutput").ap()
    zs_d = nc.dram_tensor("zs", [128, PT * 16], f32, kind="ExternalOutput").ap()
    atx_d = nc.dram_tensor("atx", [128, 256], f32, kind="ExternalOutput").ap()

    with tile.TileContext(nc) as tc:
        with (
            tc.tile_pool(name="consts", bufs=1) as cpool,
            tc.tile_pool(name="x2all", bufs=1) as x2pool,
            tc.tile_pool(name="xf", bufs=4) as xpool,
            tc.tile_pool(name="xb16", bufs=4) as xbpool,
            tc.tile_pool(name="u", bufs=3) as upool,
            tc.tile_pool(name="zsall", bufs=1) as zsapool,
            tc.tile_pool(name="an", bufs=4) as anpool,
            tc.tile_pool(name="xts", bufs=3) as xtspool,
            tc.tile_pool(name="z", bufs=4) as zpool,
            tc.tile_pool(name="atxout", bufs=1) as atxopool,
            tc.tile_pool(name="pp", bufs=2, space="PSUM") as ppool,
            tc.tile_pool(name="tp", bufs=3, space="PSUM") as tpool,
            tc.tile_pool(name="xtp", bufs=2, space="PSUM") as xtppool,
            tc.tile_pool(name="atxp", bufs=1, space="PSUM") as atxppool,
        ):
            # ---- constants -> SBUF (once) ----
            la = cpool.tile([128, 128], bf16, tag="la")
            nc.sync.dma_start(la[:], la_d[:])
            lb = cpool.tile([128, 128], bf16, tag="lb")
            nc.sync.dma_start(lb[:], lb_d[:])
            lx2 = cpool.tile([8, 128], bf16, tag="lx2")
            nc.sync.dma_start(lx2[:], lx2_d[:])
            biast = cpool.tile([128, 1], f32, tag="bias")
            nc.sync.dma_start(biast[:], bias_d[:])
            ident = cpool.tile([128, 128], bf16, tag="ident")
            nc.sync.dma_start(ident[:], id_d[:])

            # X2 chunks, hi/lo bf16 split:
            # x2a[part*4 + j, t*FC + f] = X2part[part][t*PPT + j*FC + f]
            x2a = x2pool.tile([8, PT * FC], bf16, tag="x2a")
            for part in range(2):
                nc.sync.dma_start(
                    x2a[part * 4 : part * 4 + 4].rearrange(
                        "j (t f) -> j t f", f=FC
                    ),
                    x2s[part].rearrange("(t j f) -> j t f", j=4, f=FC),
                )

            # persistent AtX accumulator (psum, f32)
            atxp = atxppool.tile([128, 256], f32, tag="atx")
            # persistent softmax-denominator collector
            zsall = zsapool.tile([128, PT * 16], f32, tag="zsall")

            import contextlib

            loop_cm = (
                tc.For_i(0, reps, 1) if reps > 1 else contextlib.nullcontext()
            )
            with loop_cm:
              for t in range(PT):
                s = t * PPT  # position offset of this P-tile

                # ---- load X: one DMA, rows (jr,c), cols (q,f); jc=2q+jr ----
                # chunk map jc = 2*jr + q: row (jr,c), col (q,f) reads a
                # contiguous 4KB run per (jr,c) -> one 3-dim DMA per P-tile
                xfull = xpool.tile([128, 2 * FC], f32, tag="xfull")
                nc.sync.dma_start(
                    xfull[:],
                    xs[:, s : s + 4 * FC].rearrange("c (jr m) -> jr c m", jr=2),
                )

                # ---- cast X to bf16 on the otherwise-idle gpsimd ----
                xf16 = xbpool.tile([128, 2 * FC], bf16, tag="xf16")
                if variant == "nocastpool":
                    nc.vector.tensor_copy(xf16[:, 0:FC], xfull[:, 0:FC])
                    nc.scalar.copy(xf16[:, FC : 2 * FC], xfull[:, FC : 2 * FC])
                else:
                    nc.gpsimd.tensor_copy(xf16[:, 0:FC], xfull[:, 0:FC])
                    nc.gpsimd.tensor_copy(
                        xf16[:, FC : 2 * FC], xfull[:, FC : 2 * FC]
                    )  # two halves so mmA can start after the first

                # ---- logits psum: P = scale*(X2 - 2 XC) packed (jc,k) x f ----
                p = ppool.tile([128, FC], f32, tag="p")
                nc.tensor.matmul(p[:], la[:], xf16[:, 0:FC], start=True, stop=False)
                nc.tensor.matmul(
                    p[:], lb[:], xf16[:, FC : 2 * FC], start=False, stop=False
                )
                nc.tensor.matmul(
                    p[:],
                    lx2[:],
                    x2a[:, t * FC : (t + 1) * FC],
                    start=False, stop=True,
                )

                # ---- U = exp(P + scale*C2) (bf16, packed KN) ----
                ut = upool.tile([128, FC], bf16, tag="u")
                u = ut[:]
                nc.scalar.activation(
                    u, p[:], mybir.ActivationFunctionType.Exp,
                    bias=biast[:, 0:1], scale=1.0,
                )
                # coefA numerator out (host divides by Z)
                nc.scalar.dma_start(ub[:, t * FC : (t + 1) * FC], ut[:])

                # ---- transpose U -> NK layout T[f, (g,jc,k)] ----
                tt = tpool.tile([128, FC], bf16, tag="tt")
                for g in range(4):
                    nc.tensor.transpose(
                        tt[:, g * 128 : (g + 1) * 128],
                        u[:, g * 128 : (g + 1) * 128],
                        ident[:],
                    )

                # ---- softmax denominators ----
                t3 = tt[:].rearrange("p (gj k) -> p gj k", k=K)
                zst = zsall[:, t * 16 : (t + 1) * 16]
                nc.vector.tensor_reduce(
                    zst, t3, axis=mybir.AxisListType.X, op=mybir.AluOpType.add
                )
                zr = zpool.tile([128, 16], f32, tag="zr")
                nc.vector.reciprocal(zr[:], zst)
                zrb = zpool.tile([128, 16], bf16, tag="zrb")
                nc.vector.tensor_copy(zrb[:], zr[:])

                # ---- A_nk = T * (1/Z) broadcast ----
                an = anpool.tile([128, FC], bf16, tag="an")
                zrbc = zrb[:].unsqueeze(2).broadcast_to([128, 16, K])
                nc.vector.tensor_tensor(
                    an[:].rearrange("p (gj k) -> p gj k", k=K),
                    t3,
                    zrbc,
                    op=mybir.AluOpType.mult,
                )

                # ---- X^T slabs + AtX accumulation ----
                # all 8 transposes land in ONE (128,1024)bf16 = 1-bank psum
                # tile; a single escape copy then feeds 4 AtX matmuls
                if variant != "noatx":
                    xtp = xtppool.tile([128, 1024], bf16, tag="xtp")
                    for g in range(4):
                        nc.tensor.transpose(
                            xtp[:, g * 256 : g * 256 + 128],
                            xf16[:, g * 128 : (g + 1) * 128],
                            ident[:],
                        )
                        nc.tensor.transpose(
                            xtp[:, g * 256 + 128 : (g + 1) * 256],
                            xf16[:, FC + g * 128 : FC + (g + 1) * 128],
                            ident[:],
                        )
                    xts = xtspool.tile([128, 1024], bf16, tag="xts")
                    if t % 2 == 0:
                        nc.vector.tensor_copy(xts[:], xtp[:])
                    else:
                        nc.scalar.copy(xts[:], xtp[:])
                    for g in range(4):
                        nc.tensor.matmul(
                            atxp[:],
                            an[:, g * 128 : (g + 1) * 128],
                            xts[:, g * 256 : (g + 1) * 256],
                            start=(t == 0 and g == 0),
                            stop=(t == PT - 1 and g == 3),
                            skip_group_check=True,
                        )

            nc.sync.dma_start(zs_d[:], zsall[:])

            # ---- drain AtX ----
            atxs = atxopool.tile([128, 256], f32, tag="atxs")
            if variant == "noatx":
                nc.vector.memset(atxs[:], 0.0)
            else:
                nc.scalar.copy(atxs[:], atxp[:])
            nc.sync.dma_start(atx_d[:], atxs[:])

    nc.compile()
    return nc


def _get_program():
    if "nc" not in _COMPILED:
        _COMPILED["nc"] = _build_program()
    return _COMPILED["nc"]


def _host_prep(X, codewords, scale):
    """Build per-core input maps."""
    cw = np.asarray(codewords, np.float32)
    sc = np.asarray(scale, np.float32)
    bf = ml_dtypes.bfloat16
    M = (-2.0 * cw * sc[:, None]).T.astype(bf)  # (C, K)
    # chunk map jc = 2*jr + q: mmA (q=0) covers jc in {0,2}, mmB jc in {1,3}
    la = np.zeros((128, 128), bf)
    lb = np.zeros((128, 128), bf)
    la[0:64, 0:32] = M       # jr=0 -> jc=0
    la[64:128, 64:96] = M    # jr=1 -> jc=2
    lb[0:64, 32:64] = M      # jr=0 -> jc=1
    lb[64:128, 96:128] = M   # jr=1 -> jc=3
    lx2 = np.zeros((8, 128), bf)
    for part in range(2):
        for j in range(4):
            lx2[part * 4 + j, j * 32 : (j + 1) * 32] = sc.astype(bf)
    c2 = (cw * cw).sum(axis=1)
    biasv = (sc * c2).astype(np.float32)
    bias = np.tile(biasv, 4)[:, None].astype(np.float32)
    ident = np.eye(128, dtype=ml_dtypes.bfloat16)

    Xr = np.asarray(X, np.float32).reshape(B, C, N)
    in_maps = []
    for core in range(NCORES):
        b, h = core // 2, core % 2
        xsl = np.ascontiguousarray(Xr[b, :, h * NSH : (h + 1) * NSH])
        x2f = np.einsum("cn,cn->n", xsl, xsl).astype(np.float32)
        x2hi = x2f.astype(bf)
        x2lo = (x2f - x2hi.astype(np.float32)).astype(bf)
        x2 = np.stack([x2hi, x2lo], axis=0)
        in_maps.append(
            {
                "xs": xsl,
                "x2s": x2,
                "la": la,
                "lb": lb,
                "lx2": lx2,
                "bias": bias,
                "ident": ident,
            }
        )
    return in_maps


def _assemble(results, codewords):
    cw = np.asarray(codewords, np.float32)
    coefA = np.empty((B, K, N), np.float32)
    E = np.empty((B, K, C), np.float32)
    atx_by_core = []
    for core in range(NCORES):
        b, h = core // 2, core % 2
        r = results[core]
        U4 = np.asarray(r["ub"]).astype(np.float32).reshape(4, K, PT, FC)
        Zt = np.asarray(r["zs"]).reshape(128, PT, 4, 4)  # [f, t, g, jc]
        Z = Zt.transpose(1, 3, 2, 0).reshape(NSH)  # n = t*2048+jc*512+g*128+f
        A = U4.transpose(1, 2, 0, 3).reshape(K, NSH) / Z[None, :]
        coefA[b, :, h * NSH : (h + 1) * NSH] = A
        atx = np.asarray(r["atx"])  # (128, 256)
        atx_p = np.zeros((K, C), np.float64)
        pos = [0, 2, 1, 3]  # XTp column-block of chunk jc under jc = 2*jr + q
        for jc in range(4):
            atx_p += atx[jc * 32 : (jc + 1) * 32, pos[jc] * 64 : pos[jc] * 64 + 64]
        atx_by_core.append(atx_p)

    for b in range(B):
        atx_b = atx_by_core[2 * b] + atx_by_core[2 * b + 1]
        asum = coefA[b].sum(axis=1, dtype=np.float64)  # (K,)
        E[b] = (atx_b - asum[:, None] * cw.astype(np.float64)).astype(np.float32)

    return E, coefA.reshape(B, K, D, H, W)


def kernel(X, codewords, scale):
    global last_exec_time_ns
    nc = _get_program()
    in_maps = _host_prep(X, codewords, scale)
    trace = bool(int(os.environ.get("KERNEL_TRACE", "0")))
    try:
        res = run_bass_kernel_spmd(
            nc, in_maps, core_ids=list(range(NCORES)), trace=trace
        )
    except (ModuleNotFoundError, ImportError):
        res = run_bass_kernel_spmd(
            nc, in_maps, core_ids=list(range(NCORES)), trace=False
        )
    last_exec_time_ns = res.exec_time_ns
    return _assemble(res.results, codewords)


if __name__ == "__main__":
    # quick numeric self-check against a numpy reference
    rng = np.random.default_rng(0)
    X = rng.standard_normal((B, C, D, H, W), dtype=np.float32)
    cw = (rng.random((K, C), dtype=np.float32) - 0.5) * 2 / np.sqrt(K * C)
    sc = -rng.random(K, dtype=np.float32)
    E, coefA = kernel(X, cw, sc)
    print("E", E.shape, "coefA", coefA.shape, "exec_ns", last_exec_time_ns)


# revision 22
# speedup vs baseline: 1.3988x; 1.3988x over previous
"""Trainium2 Bass kernel for nn_Encoding3D (vq_codebook encoding layer).

Computes, for X (B,C,D,H,W), codewords (K,C), scale (K,):
    logits[b,n,k] = scale_k * (|x_bn|^2 + |c_k|^2 - 2 x_bn.c_k)
    A = softmax_k(logits);  coefA = A^T reshaped (B,K,D,H,W)
    E[b,k,:] = sum_n A[b,n,k] * (x_bn - c_k)
Returns (E, coefA) like the reference.

Sharding: 8 cores, each handles half of one batch's N = D*H*W positions.
Device pipeline per core (positions in P-tiles of 2048 = 4 chunks x 512):
  - 3 accumulating bf16 matmuls produce packed-KN logits psum
    P[(jc,k), f] = scale_k*(X2 - 2 XC) (codebook/scale folded into weights;
    X2 precomputed on host, split hi/lo bf16 for precision, injected via a
    rank-8 matmul)
  - ACT exp with per-partition bias scale_k*|c_k|^2 -> U (unnormalized
    softmax numerator, bf16, exp<=0 args so no overflow; max-subtraction
    is unnecessary since max logit ~ -few, verified vs reference)
  - PE transposes U into NK layout; DVE computes Z (sum over k), 1/Z
  - X cast to bf16, PE-transposed to X^T tiles; AtX accumulated over the
    whole shard into one persistent PSUM tile via matmuls
    lhsT = A_nk-slab (128f x 128(jc,k)), rhs = X^T-slab (128f x 256(jc,c))
  - outputs: U (packed KN, bf16), Z (f32), AtX partials (f32)
Host: divides U by Z (coefA), sums Asum from coefA, E = AtX - Asum*cw.
"""

import os
import sys

sys.path.insert(0, "/opt/trn_rl_repo")

import numpy as np
import ml_dtypes

import concourse.bass as bass
import concourse.bacc as bacc
import concourse.tile as tile
from concourse import mybir
from concourse.bass_utils import run_bass_kernel_spmd

# Problem dims (hardcoded per contract)
B, C, D, H, W = 4, 64, 32, 64, 64
K = 32
N = D * H * W  # 131072
NCORES = 8
NSH = B * N // NCORES  # 65536 positions per core (half a batch)
FC = 512  # psum free columns
PPT = 4 * FC  # positions per P-tile (4 chunks of FC)
PT = NSH // PPT  # 32 P-tiles

f32 = mybir.dt.float32
f32r = mybir.dt.float32r
bf16 = mybir.dt.bfloat16

_COMPILED = {}

last_exec_time_ns = None


def _build_program(reps=1, variant="full"):
    nc = bacc.Bacc("TRN2", target_bir_lowering=False, debug=False)

    xs = nc.dram_tensor("xs", [C, NSH], f32, kind="ExternalInput").ap()
    x2s = nc.dram_tensor("x2s", [2, NSH], bf16, kind="ExternalInput").ap()
    la_d = nc.dram_tensor("la", [128, 128], bf16, kind="ExternalInput").ap()
    lb_d = nc.dram_tensor("lb", [128, 128], bf16, kind="ExternalInput").ap()
    lx2_d = nc.dram_tensor("lx2", [8, 128], bf16, kind="ExternalInput").ap()
    bias_d = nc.dram_tensor("bias", [128, 1], f32, kind="ExternalInput").ap()
    id_d = nc.dram_tensor("ident", [128, 128], bf16, kind="ExternalInput").ap()

    ub = nc.dram_tensor("ub", [128, PT * FC], bf16, kind="ExternalOutput").ap()
    zs_d = nc.dram_tensor("zs", [128, PT * 16], f32, kind="ExternalOutput").ap()
    atx_d = nc.dram_tensor("atx", [128, 256], f32, kind="ExternalOutput").ap()

    with tile.TileContext(nc) as tc:
        with (
            tc.tile_pool(name="consts", bufs=1) as cpool,
            tc.tile_pool(name="x2all", bufs=1) as x2pool,
            tc.tile_pool(name="xf", bufs=4) as xpool,
            tc.tile_pool(name="xb16", bufs=4) as xbpool,
            tc.tile_pool(name="u", bufs=3) as upool,
            tc.tile_pool(name="zsall", bufs=1) as zsapool,
            tc.tile_pool(name="an", bufs=4) as anpool,
            tc.tile_pool(name="xts", bufs=6) as xtspool,
            tc.tile_pool(name="z", bufs=4) as zpool,
            tc.tile_pool(name="atxout", bufs=1) as atxopool,
            tc.tile_pool(name="pp", bufs=2, space="PSUM") as ppool,
            tc.tile_pool(name="tp", bufs=3, space="PSUM") as tpool,
            tc.tile_pool(name="xtp", bufs=2, space="PSUM") as xtppool,
            tc.tile_pool(name="atxp", bufs=1, space="PSUM") as atxppool,
        ):
            # ---- constants -> SBUF (once) ----
            la = cpool.tile([128, 128], bf16, tag="la")
            nc.sync.dma_start(la[:], la_d[:])
            lb = cpool.tile([128, 128], bf16, tag="lb")
            nc.sync.dma_start(lb[:], lb_d[:])
            lx2 = cpool.tile([8, 128], bf16, tag="lx2")
            nc.sync.dma_start(lx2[:], lx2_d[:])
            biast = cpool.tile([128, 1], f32, tag="bias")
            nc.sync.dma_start(biast[:], bias_d[:])
            ident = cpool.tile([128, 128], bf16, tag="ident")
            nc.sync.dma_start(ident[:], id_d[:])

            # X2 chunks, hi/lo bf16 split:
            # x2a[part*4 + j, t*FC + f] = X2part[part][t*PPT + j*FC + f]
            x2a = x2pool.tile([8, PT * FC], bf16, tag="x2a")
            for part in range(2):
                nc.sync.dma_start(
                    x2a[part * 4 : part * 4 + 4].rearrange(
                        "j (t f) -> j t f", f=FC
                    ),
                    x2s[part].rearrange("(t j f) -> j t f", j=4, f=FC),
                )

            # persistent AtX accumulator (psum, f32)
            atxp = atxppool.tile([128, 256], f32, tag="atx")
            # persistent softmax-denominator collector
            zsall = zsapool.tile([128, PT * 16], f32, tag="zsall")

            import contextlib

            loop_cm = (
                tc.For_i(0, reps, 1) if reps > 1 else contextlib.nullcontext()
            )
            with loop_cm:
              for t in range(PT):
                s = t * PPT  # position offset of this P-tile

                # ---- load X: one DMA, rows (jr,c), cols (q,f); jc=2q+jr ----
                # chunk map jc = 2*jr + q: row (jr,c), col (q,f) reads a
                # contiguous 4KB run per (jr,c) -> one 3-dim DMA per P-tile
                xfull = xpool.tile([128, 2 * FC], f32, tag="xfull")
                nc.sync.dma_start(
                    xfull[:],
                    xs[:, s : s + 4 * FC].rearrange("c (jr m) -> jr c m", jr=2),
                )

                # ---- cast X to bf16 on the otherwise-idle gpsimd ----
                xf16 = xbpool.tile([128, 2 * FC], bf16, tag="xf16")
                if variant == "nocastpool":
                    nc.vector.tensor_copy(xf16[:, 0:FC], xfull[:, 0:FC])
                    nc.scalar.copy(xf16[:, FC : 2 * FC], xfull[:, FC : 2 * FC])
                else:
                    nc.gpsimd.tensor_copy(xf16[:, 0:FC], xfull[:, 0:FC])
                    nc.gpsimd.tensor_copy(
                        xf16[:, FC : 2 * FC], xfull[:, FC : 2 * FC]
                    )  # two halves so mmA can start after the first

                # ---- logits psum: P = scale*(X2 - 2 XC) packed (jc,k) x f ----
                p = ppool.tile([128, FC], f32, tag="p")
                nc.tensor.matmul(p[:], la[:], xf16[:, 0:FC], start=True, stop=False)
                nc.tensor.matmul(
                    p[:], lb[:], xf16[:, FC : 2 * FC], start=False, stop=False
                )
                nc.tensor.matmul(
                    p[:],
                    lx2[:],
                    x2a[:, t * FC : (t + 1) * FC],
                    start=False, stop=True,
                )

                # ---- U = exp(P + scale*C2) (bf16, packed KN) ----
                ut = upool.tile([128, FC], bf16, tag="u")
                u = ut[:]
                nc.scalar.activation(
                    u, p[:], mybir.ActivationFunctionType.Exp,
                    bias=biast[:, 0:1], scale=1.0,
                )
                # coefA numerator out (host divides by Z)
                nc.scalar.dma_start(ub[:, t * FC : (t + 1) * FC], ut[:])

                # ---- transpose U -> NK layout T[f, (g,jc,k)] ----
                tt = tpool.tile([128, FC], bf16, tag="tt")
                for g in range(4):
                    nc.tensor.transpose(
                        tt[:, g * 128 : (g + 1) * 128],
                        u[:, g * 128 : (g + 1) * 128],
                        ident[:],
                    )

                # ---- softmax denominators ----
                t3 = tt[:].rearrange("p (gj k) -> p gj k", k=K)
                zst = zsall[:, t * 16 : (t + 1) * 16]
                nc.vector.tensor_reduce(
                    zst, t3, axis=mybir.AxisListType.X, op=mybir.AluOpType.add
                )
                zr = zpool.tile([128, 16], f32, tag="zr")
                nc.vector.reciprocal(zr[:], zst)
                zrb = zpool.tile([128, 16], bf16, tag="zrb")
                nc.vector.tensor_copy(zrb[:], zr[:])

                # ---- A_nk = T * (1/Z) broadcast ----
                an = anpool.tile([128, FC], bf16, tag="an")
                zrbc = zrb[:].unsqueeze(2).broadcast_to([128, 16, K])
                nc.vector.tensor_tensor(
                    an[:].rearrange("p (gj k) -> p gj k", k=K),
                    t3,
                    zrbc,
                    op=mybir.AluOpType.mult,
                )

                # ---- X^T slabs + AtX accumulation ----
                for g in range(4 if variant != "noatx" else 0):
                    xtp = xtppool.tile([128, 256], bf16, tag="xtp")
                    nc.tensor.transpose(
                        xtp[:, 0:128], xf16[:, g * 128 : (g + 1) * 128], ident[:]
                    )
                    nc.tensor.transpose(
                        xtp[:, 128:256],
                        xf16[:, FC + g * 128 : FC + (g + 1) * 128],
                        ident[:],
                    )
                    xts = xtspool.tile([128, 256], bf16, tag="xts")
                    if g == 0:
                        nc.vector.tensor_copy(xts[:], xtp[:])
                    else:
                        nc.scalar.copy(xts[:], xtp[:])

                    nc.tensor.matmul(
                        atxp[:],
                        an[:, g * 128 : (g + 1) * 128],
                        xts[:],
                        start=(t == 0 and g == 0),
                        stop=(t == PT - 1 and g == 3),
                        skip_group_check=True,
                    )

            nc.sync.dma_start(zs_d[:], zsall[:])

            # ---- drain AtX ----
            atxs = atxopool.tile([128, 256], f32, tag="atxs")
            if variant == "noatx":
                nc.vector.memset(atxs[:], 0.0)
            else:
                nc.scalar.copy(atxs[:], atxp[:])
            nc.sync.dma_start(atx_d[:], atxs[:])

    nc.compile()
    return nc


def _get_program():
    if "nc" not in _COMPILED:
        _COMPILED["nc"] = _build_program()
    return _COMPILED["nc"]


def _host_prep(X, codewords, scale):
    """Build per-core input maps."""
    cw = np.asarray(codewords, np.float32)
    sc = np.asarray(scale, np.float32)
    bf = ml_dtypes.bfloat16
    M = (-2.0 * cw * sc[:, None]).T.astype(bf)  # (C, K)
    # chunk map jc = 2*jr + q: mmA (q=0) covers jc in {0,2}, mmB jc in {1,3}
    la = np.zeros((128, 128), bf)
    lb = np.zeros((128, 128), bf)
    la[0:64, 0:32] = M       # jr=0 -> jc=0
    la[64:128, 64:96] = M    # jr=1 -> jc=2
    lb[0:64, 32:64] = M      # jr=0 -> jc=1
    lb[64:128, 96:128] = M   # jr=1 -> jc=3
    lx2 = np.zeros((8, 128), bf)
    for part in range(2):
        for j in range(4):
            lx2[part * 4 + j, j * 32 : (j + 1) * 32] = sc.astype(bf)
    c2 = (cw * cw).sum(axis=1)
    biasv = (sc * c2).astype(np.float32)
    bias = np.tile(biasv, 4)[:, None].astype(np.float32)
    ident = np.eye(128, dtype=ml_dtypes.bfloat16)

    Xr = np.asarray(X, np.float32).reshape(B, C, N)
    in_maps = []
    for core in range(NCORES):
        b, h = core // 2, core % 2
        xsl = np.ascontiguousarray(Xr[b, :, h * NSH : (h + 1) * NSH])
        x2f = np.einsum("cn,cn->n", xsl, xsl).astype(np.float32)
        x2hi = x2f.astype(bf)
        x2lo = (x2f - x2hi.astype(np.float32)).astype(bf)
        x2 = np.stack([x2hi, x2lo], axis=0)
        in_maps.append(
            {
                "xs": xsl,
                "x2s": x2,
                "la": la,
                "lb": lb,
                "lx2": lx2,
                "bias": bias,
                "ident": ident,
            }
        )
    return in_maps


def _assemble(results, codewords):
    cw = np.asarray(codewords, np.float32)
    coefA = np.empty((B, K, N), np.float32)
    E = np.empty((B, K, C), np.float32)
    atx_by_core = []
    for core in range(NCORES):
        b, h = core // 2, core % 2
        r = results[core]
        U4 = np.asarray(r["ub"]).astype(np.float32).reshape(4, K, PT, FC)
        Zt = np.asarray(r["zs"]).reshape(128, PT, 4, 4)  # [f, t, g, jc]
        Z = Zt.transpose(1, 3, 2, 0).reshape(NSH)  # n = t*2048+jc*512+g*128+f
        A = U4.transpose(1, 2, 0, 3).reshape(K, NSH) / Z[None, :]
        coefA[b, :, h * NSH : (h + 1) * NSH] = A
        atx = np.asarray(r["atx"])  # (128, 256)
        atx_p = np.zeros((K, C), np.float64)
        pos = [0, 2, 1, 3]  # XTp column-block of chunk jc under jc = 2*jr + q
        for jc in range(4):
            atx_p += atx[jc * 32 : (jc + 1) * 32, pos[jc] * 64 : pos[jc] * 64 + 64]
        atx_by_core.append(atx_p)

    for b in range(B):
        atx_b = atx_by_core[2 * b] + atx_by_core[2 * b + 1]
        asum = coefA[b].sum(axis=1, dtype=np.float64)  # (K,)
        E[b] = (atx_b - asum[:, None] * cw.astype(np.float64)).astype(np.float32)

    return E, coefA.reshape(B, K, D, H, W)


def kernel(X, codewords, scale):
    global last_exec_time_ns
    nc = _get_program()
    in_maps = _host_prep(X, codewords, scale)
    trace = bool(int(os.environ.get("KERNEL_TRACE", "0")))
    try:
        res = run_bass_kernel_spmd(
            nc, in_maps, core_ids=list(range(NCORES)), trace=trace
        )
    except (ModuleNotFoundError, ImportError):
        res = run_bass_kernel_spmd(
            nc, in_maps, core_ids=list(range(NCORES)), trace=False
        )
    last_exec_time_ns = res.exec_time_ns
    return _assemble(res.results, codewords)


if __name__ == "__main__":
    # quick numeric self-check against a numpy reference
    rng = np.random.default_rng(0)
    X = rng.standard_normal((B, C, D, H, W), dtype=np.float32)
    cw = (rng.random((K, C), dtype=np.float32) - 0.5) * 2 / np.sqrt(K * C)
    sc = -rng.random(K, dtype=np.float32)
    E, coefA = kernel(X, cw, sc)
    print("E", E.shape, "coefA", coefA.shape, "exec_ns", last_exec_time_ns)


# revision 23
# speedup vs baseline: 1.4504x; 1.0369x over previous
"""Trainium2 Bass kernel for nn_Encoding3D (vq_codebook encoding layer).

Computes, for X (B,C,D,H,W), codewords (K,C), scale (K,):
    logits[b,n,k] = scale_k * (|x_bn|^2 + |c_k|^2 - 2 x_bn.c_k)
    A = softmax_k(logits);  coefA = A^T reshaped (B,K,D,H,W)
    E[b,k,:] = sum_n A[b,n,k] * (x_bn - c_k)
Returns (E, coefA) like the reference.

Sharding: 8 cores, each handles half of one batch's N = D*H*W positions.
Device pipeline per core (positions in P-tiles of 2048 = 4 chunks x 512):
  - 3 accumulating bf16 matmuls produce packed-KN logits psum
    P[(jc,k), f] = scale_k*(X2 - 2 XC) (codebook/scale folded into weights;
    X2 precomputed on host, split hi/lo bf16 for precision, injected via a
    rank-8 matmul)
  - ACT exp with per-partition bias scale_k*|c_k|^2 -> U (unnormalized
    softmax numerator, bf16, exp<=0 args so no overflow; max-subtraction
    is unnecessary since max logit ~ -few, verified vs reference)
  - PE transposes U into NK layout; DVE computes Z (sum over k), 1/Z
  - X cast to bf16, PE-transposed to X^T tiles; AtX accumulated over the
    whole shard into one persistent PSUM tile via matmuls
    lhsT = A_nk-slab (128f x 128(jc,k)), rhs = X^T-slab (128f x 256(jc,c))
  - outputs: U (packed KN, bf16), Z (f32), AtX partials (f32)
Host: divides U by Z (coefA), sums Asum from coefA, E = AtX - Asum*cw.
"""

import os
import sys

sys.path.insert(0, "/opt/trn_rl_repo")

import numpy as np
import ml_dtypes

import concourse.bass as bass
import concourse.bacc as bacc
import concourse.tile as tile
from concourse import mybir
from concourse.bass_utils import run_bass_kernel_spmd

# Problem dims (hardcoded per contract)
B, C, D, H, W = 4, 64, 32, 64, 64
K = 32
N = D * H * W  # 131072
NCORES = 8
NSH = B * N // NCORES  # 65536 positions per core (half a batch)
FC = 512  # psum free columns
PPT = 4 * FC  # positions per P-tile (4 chunks of FC)
PT = NSH // PPT  # 32 P-tiles

f32 = mybir.dt.float32
f32r = mybir.dt.float32r
bf16 = mybir.dt.bfloat16

_COMPILED = {}

last_exec_time_ns = None


def _build_program(reps=1, variant="full"):
    nc = bacc.Bacc("TRN2", target_bir_lowering=False, debug=False)

    xs = nc.dram_tensor("xs", [C, NSH], f32, kind="ExternalInput").ap()
    x2s = nc.dram_tensor("x2s", [2, NSH], bf16, kind="ExternalInput").ap()
    la_d = nc.dram_tensor("la", [128, 128], bf16, kind="ExternalInput").ap()
    lb_d = nc.dram_tensor("lb", [128, 128], bf16, kind="ExternalInput").ap()
    lx2_d = nc.dram_tensor("lx2", [8, 128], bf16, kind="ExternalInput").ap()
    bias_d = nc.dram_tensor("bias", [128, 1], f32, kind="ExternalInput").ap()
    id_d = nc.dram_tensor("ident", [128, 128], bf16, kind="ExternalInput").ap()

    ub = nc.dram_tensor("ub", [128, PT * FC], bf16, kind="ExternalOutput").ap()
    zs_d = nc.dram_tensor("zs", [128, PT * 16], f32, kind="ExternalOutput").ap()
    atx_d = nc.dram_tensor("atx", [128, 256], f32, kind="ExternalOutput").ap()

    with tile.TileContext(nc) as tc:
        with (
            tc.tile_pool(name="consts", bufs=1) as cpool,
            tc.tile_pool(name="x2all", bufs=1) as x2pool,
            tc.tile_pool(name="xf", bufs=6) as xpool,
            tc.tile_pool(name="xb16", bufs=6) as xbpool,
            tc.tile_pool(name="u", bufs=6) as upool,
            tc.tile_pool(name="zsall", bufs=1) as zsapool,
            tc.tile_pool(name="an", bufs=6) as anpool,
            tc.tile_pool(name="xts", bufs=8) as xtspool,
            tc.tile_pool(name="z", bufs=8) as zpool,
            tc.tile_pool(name="atxout", bufs=1) as atxopool,
            tc.tile_pool(name="pp", bufs=2, space="PSUM") as ppool,
            tc.tile_pool(name="tp", bufs=3, space="PSUM") as tpool,
            tc.tile_pool(name="xtp", bufs=2, space="PSUM") as xtppool,
            tc.tile_pool(name="atxp", bufs=1, space="PSUM") as atxppool,
        ):
            # ---- constants -> SBUF (once) ----
            la = cpool.tile([128, 128], bf16, tag="la")
            nc.sync.dma_start(la[:], la_d[:])
            lb = cpool.tile([128, 128], bf16, tag="lb")
            nc.sync.dma_start(lb[:], lb_d[:])
            lx2 = cpool.tile([8, 128], bf16, tag="lx2")
            nc.sync.dma_start(lx2[:], lx2_d[:])
            biast = cpool.tile([128, 1], f32, tag="bias")
            nc.sync.dma_start(biast[:], bias_d[:])
            ident = cpool.tile([128, 128], bf16, tag="ident")
            nc.sync.dma_start(ident[:], id_d[:])

            # X2 chunks, hi/lo bf16 split:
            # x2a[part*4 + j, t*FC + f] = X2part[part][t*PPT + j*FC + f]
            x2a = x2pool.tile([8, PT * FC], bf16, tag="x2a")
            for part in range(2):
                nc.sync.dma_start(
                    x2a[part * 4 : part * 4 + 4].rearrange(
                        "j (t f) -> j t f", f=FC
                    ),
                    x2s[part].rearrange("(t j f) -> j t f", j=4, f=FC),
                )

            # persistent AtX accumulator (psum, f32)
            atxp = atxppool.tile([128, 256], f32, tag="atx")
            # persistent softmax-denominator collector
            zsall = zsapool.tile([128, PT * 16], f32, tag="zsall")

            import contextlib

            loop_cm = (
                tc.For_i(0, reps, 1) if reps > 1 else contextlib.nullcontext()
            )
            with loop_cm:
              for t in range(PT):
                s = t * PPT  # position offset of this P-tile

                # ---- load X: one DMA, rows (jr,c), cols (q,f); jc=2q+jr ----
                # chunk map jc = 2*jr + q: row (jr,c), col (q,f) reads a
                # contiguous 4KB run per (jr,c) -> one 3-dim DMA per P-tile
                xfull = xpool.tile([128, 2 * FC], f32, tag="xfull")
                nc.sync.dma_start(
                    xfull[:],
                    xs[:, s : s + 4 * FC].rearrange("c (jr m) -> jr c m", jr=2),
                )

                # ---- cast X to bf16 on the otherwise-idle gpsimd ----
                xf16 = xbpool.tile([128, 2 * FC], bf16, tag="xf16")
                if variant == "nocastpool":
                    nc.vector.tensor_copy(xf16[:, 0:FC], xfull[:, 0:FC])
                    nc.scalar.copy(xf16[:, FC : 2 * FC], xfull[:, FC : 2 * FC])
                else:
                    nc.gpsimd.tensor_copy(xf16[:, 0:FC], xfull[:, 0:FC])
                    nc.gpsimd.tensor_copy(
                        xf16[:, FC : 2 * FC], xfull[:, FC : 2 * FC]
                    )  # two halves so mmA can start after the first

                # ---- logits psum: P = scale*(X2 - 2 XC) packed (jc,k) x f ----
                p = ppool.tile([128, FC], f32, tag="p")
                nc.tensor.matmul(p[:], la[:], xf16[:, 0:FC], start=True, stop=False)
                nc.tensor.matmul(
                    p[:], lb[:], xf16[:, FC : 2 * FC], start=False, stop=False
                )
                nc.tensor.matmul(
                    p[:],
                    lx2[:],
                    x2a[:, t * FC : (t + 1) * FC],
                    start=False, stop=True,
                )

                # ---- U = exp(P + scale*C2) (bf16, packed KN) ----
                ut = upool.tile([128, FC], bf16, tag="u")
                u = ut[:]
                nc.scalar.activation(
                    u, p[:], mybir.ActivationFunctionType.Exp,
                    bias=biast[:, 0:1], scale=1.0,
                )
                # coefA numerator out (host divides by Z)
                nc.sync.dma_start(ub[:, t * FC : (t + 1) * FC], ut[:])

                # ---- transpose U -> NK layout T[f, (g,jc,k)] ----
                tt = tpool.tile([128, FC], bf16, tag="tt")
                for g in range(4):
                    nc.tensor.transpose(
                        tt[:, g * 128 : (g + 1) * 128],
                        u[:, g * 128 : (g + 1) * 128],
                        ident[:],
                    )

                # ---- softmax denominators ----
                t3 = tt[:].rearrange("p (gj k) -> p gj k", k=K)
                zst = zsall[:, t * 16 : (t + 1) * 16]
                nc.vector.tensor_reduce(
                    zst, t3, axis=mybir.AxisListType.X, op=mybir.AluOpType.add
                )
                zr = zpool.tile([128, 16], f32, tag="zr")
                nc.vector.reciprocal(zr[:], zst)
                zrb = zpool.tile([128, 16], bf16, tag="zrb")
                nc.vector.tensor_copy(zrb[:], zr[:])

                # ---- A_nk = T * (1/Z) broadcast ----
                an = anpool.tile([128, FC], bf16, tag="an")
                zrbc = zrb[:].unsqueeze(2).broadcast_to([128, 16, K])
                nc.vector.tensor_tensor(
                    an[:].rearrange("p (gj k) -> p gj k", k=K),
                    t3,
                    zrbc,
                    op=mybir.AluOpType.mult,
                )

                # ---- X^T slabs + AtX accumulation ----
                for g in range(4 if variant != "noatx" else 0):
                    xtp = xtppool.tile([128, 256], bf16, tag="xtp")
                    nc.tensor.transpose(
                        xtp[:, 0:128], xf16[:, g * 128 : (g + 1) * 128], ident[:]
                    )
                    nc.tensor.transpose(
                        xtp[:, 128:256],
                        xf16[:, FC + g * 128 : FC + (g + 1) * 128],
                        ident[:],
                    )
                    xts = xtspool.tile([128, 256], bf16, tag="xts")
                    if g == 0:
                        nc.vector.tensor_copy(xts[:], xtp[:])
                    else:
                        nc.scalar.copy(xts[:], xtp[:])

                    nc.tensor.matmul(
                        atxp[:],
                        an[:, g * 128 : (g + 1) * 128],
                        xts[:],
                        start=(t == 0 and g == 0),
                        stop=(t == PT - 1 and g == 3),
                        skip_group_check=True,
                    )

            nc.sync.dma_start(zs_d[:], zsall[:])

            # ---- drain AtX ----
            atxs = atxopool.tile([128, 256], f32, tag="atxs")
            if variant == "noatx":
                nc.vector.memset(atxs[:], 0.0)
            else:
                nc.scalar.copy(atxs[:], atxp[:])
            nc.sync.dma_start(atx_d[:], atxs[:])

    nc.compile()
    return nc


def _get_program():
    if "nc" not in _COMPILED:
        _COMPILED["nc"] = _build_program()
    return _COMPILED["nc"]


def _host_prep(X, codewords, scale):
    """Build per-core input maps."""
    cw = np.asarray(codewords, np.float32)
    sc = np.asarray(scale, np.float32)
    bf = ml_dtypes.bfloat16
    M = (-2.0 * cw * sc[:, None]).T.astype(bf)  # (C, K)
    # chunk map jc = 2*jr + q: mmA (q=0) covers jc in {0,2}, mmB jc in {1,3}
    la = np.zeros((128, 128), bf)
    lb = np.zeros((128, 128), bf)
    la[0:64, 0:32] = M       # jr=0 -> jc=0
    la[64:128, 64:96] = M    # jr=1 -> jc=2
    lb[0:64, 32:64] = M      # jr=0 -> jc=1
    lb[64:128, 96:128] = M   # jr=1 -> jc=3
    lx2 = np.zeros((8, 128), bf)
    for part in range(2):
        for j in range(4):
            lx2[part * 4 + j, j * 32 : (j + 1) * 32] = sc.astype(bf)
    c2 = (cw * cw).sum(axis=1)
    biasv = (sc * c2).astype(np.float32)
    bias = np.tile(biasv, 4)[:, None].astype(np.float32)
    ident = np.eye(128, dtype=ml_dtypes.bfloat16)

    Xr = np.asarray(X, np.float32).reshape(B, C, N)
    in_maps = []
    for core in range(NCORES):
        b, h = core // 2, core % 2
        xsl = np.ascontiguousarray(Xr[b, :, h * NSH : (h + 1) * NSH])
        x2f = np.einsum("cn,cn->n", xsl, xsl).astype(np.float32)
        x2hi = x2f.astype(bf)
        x2lo = (x2f - x2hi.astype(np.float32)).astype(bf)
        x2 = np.stack([x2hi, x2lo], axis=0)
        in_maps.append(
            {
                "xs": xsl,
                "x2s": x2,
                "la": la,
                "lb": lb,
                "lx2": lx2,
                "bias": bias,
                "ident": ident,
            }
        )
    return in_maps


def _assemble(results, codewords):
    cw = np.asarray(codewords, np.float32)
    coefA = np.empty((B, K, N), np.float32)
    E = np.empty((B, K, C), np.float32)
    atx_by_core = []
    for core in range(NCORES):
        b, h = core // 2, core % 2
        r = results[core]
        U4 = np.asarray(r["ub"]).astype(np.float32).reshape(4, K, PT, FC)
        Zt = np.asarray(r["zs"]).reshape(128, PT, 4, 4)  # [f, t, g, jc]
        Z = Zt.transpose(1, 3, 2, 0).reshape(NSH)  # n = t*2048+jc*512+g*128+f
        A = U4.transpose(1, 2, 0, 3).reshape(K, NSH) / Z[None, :]
        coefA[b, :, h * NSH : (h + 1) * NSH] = A
        atx = np.asarray(r["atx"])  # (128, 256)
        atx_p = np.zeros((K, C), np.float64)
        pos = [0, 2, 1, 3]  # XTp column-block of chunk jc under jc = 2*jr + q
        for jc in range(4):
            atx_p += atx[jc * 32 : (jc + 1) * 32, pos[jc] * 64 : pos[jc] * 64 + 64]
        atx_by_core.append(atx_p)

    for b in range(B):
        atx_b = atx_by_core[2 * b] + atx_by_core[2 * b + 1]
        asum = coefA[b].sum(axis=1, dtype=np.float64)  # (K,)
        E[b] = (atx_b - asum[:, None] * cw.astype(np.float64)).astype(np.float32)

    return E, coefA.reshape(B, K, D, H, W)


def kernel(X, codewords, scale):
    global last_exec_time_ns
    nc = _get_program()
    in_maps = _host_prep(X, codewords, scale)
    trace = bool(int(os.environ.get("KERNEL_TRACE", "0")))
    try:
        res = run_bass_kernel_spmd(
            nc, in_maps, core_ids=list(range(NCORES)), trace=trace
        )
    except (ModuleNotFoundError, ImportError):
        res = run_bass_kernel_spmd(
            nc, in_maps, core_ids=list(range(NCORES)), trace=False
        )
    last_exec_time_ns = res.exec_time_ns
    return _assemble(res.results, codewords)


if __name__ == "__main__":
    # quick numeric self-check against a numpy reference
    rng = np.random.default_rng(0)
    X = rng.standard_normal((B, C, D, H, W), dtype=np.float32)
    cw = (rng.random((K, C), dtype=np.float32) - 0.5) * 2 / np.sqrt(K * C)
    sc = -rng.random(K, dtype=np.float32)
    E, coefA = kernel(X, cw, sc)
    print("E", E.shape, "coefA", coefA.shape, "exec_ns", last_exec_time_ns)
